# revision 29
# baseline (speedup 1.0000x reference)
"""BRD4KANModel Trainium2 kernel, v2.

Data-parallel over batch across 8 NeuronCores (512 rows each, weights
replicated). On-chip layout is feature-major (h^T: features on partitions,
batch on the free dim); every layer's matmul output [out_feat, batch]
feeds the next layer directly. All matmul operands are fp16 (cast from
f32 during the SWDGE DMA); weights are transposed on-chip by the PE
(128x128 is_transpose matmuls into fp16 PSUM banks of 8 tiles each).

B-spline bases use the symmetric cardinal form: B_c(x) = M4(t),
t = (x - grid[c+2])/h, M4(t) = (1/6)(2-|t|)+^3 - (4/6)(1-|t|)+^3.
With U = a|t| (a = 6^(-1/3), one ACT Abs per channel), both truncated
powers come from min/subtract chains whose constants are channel-
independent, so all DVE steps run batched over the 6 channels at fp16
2x/4x DVE rates:
    A  = min(U, 2a) - 2a          (= -a(2-|t|)+,  tensor_scalar 4x)
    Bq = min(U, a)  - a           (= -a(1-|t|)+,  tensor_scalar 4x)
    A3 = (A*A)*A,  B3 = (2Bq)^2*Bq  (squares+cubes, tensor_tensor 2x)
    bases = B3 - A3               (on Pool; it is idle otherwise)
Max abs error vs the Cox-de-Boor reference is ~2.7e-3 in fp16, and the
spline weights are small (~2e-3 rms after scaler), so the end-to-end
error stays ~1e-3.

The per-(out,in) spline scaler multiply rides the PSUM->SBUF evacuation
of the weight transposes: the scaler strip is itself PE-transposed to
[in, out] layout, and the spline-weight evac becomes one fp16
tensor_tensor multiply per PSUM bank instead of a separate full pass
over all 63M spline weights.

This walrus build accepts only ONE semaphore wait per instruction, while
Tile's scheduler attaches several; _split_waits() post-processes the BIR
JSON, hoisting excess waits onto NoOps inserted just before each
instruction on the same engine.
"""

import json
import os

import numpy as np

import concourse.bass as bass
import concourse.mybir as mybir
import concourse.tile as tile
from concourse.masks import make_identity

F32 = mybir.dt.float32
F16 = mybir.dt.float16
F8 = mybir.dt.float8e4
AF = mybir.ActivationFunctionType
OP = mybir.AluOpType
PM = mybir.MatmulPerfMode

# fp8 pre-scales: spline weights (sw*sc ~ 2e-3 rms) sit below the e4m3
# subnormal floor, so they are scaled by WS at evacuation. The bases are
# scaled by BS=32 (folded into the cube-chain constants) so their fp8
# quantization RESIDUAL lands in e4m3's normal range instead of flushing
# to zero. h therefore flows through the network multiplied by
# WS_H = WS*BS; the ACT input scales of silu/abs (and the head epilogue)
# divide it back out for free.
WS = 128.0
WS_H = WS

N_CORES = 8
BATCH = 4096
B = BATCH // N_CORES  # 512 per core
D = 2048
WIDTHS = [2048, 2048, 1024]
COEFF = 6
GRID_SIZE = 3
SPLINE_ORDER = 3
HG = 2.0 / GRID_SIZE
GRID = [m * HG - 1.0 - SPLINE_ORDER * HG for m in range(GRID_SIZE + 2 * SPLINE_ORDER + 1)]
ALPHA = 6.0 ** (-1.0 / 3.0)
SC_U = ALPHA / HG
BIAS_C = [-ALPHA * GRID[c + 2] / HG for c in range(COEFF)]

CH = 256          # spline i-chunk (features per weight DMA chunk)


def _split_waits(bir_bytes: bytes, keep: int = 1) -> bytes:
    d = json.loads(bir_bytes)
    for f in d["functions"]:
        for bb in f["blocks"]:
            new_insts = []
            for inst in bb["instructions"]:
                si = inst.get("sync_info")
                waits = (si or {}).get("on_wait") or []
                if len(waits) > keep:
                    extra = waits[:-keep]
                    inst["sync_info"]["on_wait"] = waits[-keep:]
                    for ci in range(0, len(extra), keep):
                        new_insts.append({
                            "name": f"{inst['name']}-w{ci}",
                            "opcode": "NoOp",
                            "engine": inst["engine"],
                            "ins": [],
                            "outs": [],
                            "debug": inst.get("debug"),
                            "sync_info": {"on_update": [],
                                          "on_wait": extra[ci:ci + keep]},
                        })
                new_insts.append(inst)
            bb["instructions"] = new_insts
    return json.dumps(d).encode()


def _patch_json(nc):
    orig = nc.to_json_bytes

    def patched():
        return _split_waits(orig())

    nc.to_json_bytes = patched
    return nc


def build(stage=99):
    nc = bass.Bass()
    x = nc.dram_tensor("x", [B, D], F32, kind="ExternalInput")
    mult_w = nc.dram_tensor("mult_w", [2 * D, D], F32, kind="ExternalInput")
    mult_b = nc.dram_tensor("mult_b", [2 * D], F32, kind="ExternalInput")
    kan = []
    dims = [D] + WIDTHS
    for l in range(3):
        fo = dims[l + 1]
        kan.append((
            nc.dram_tensor(f"base_w{l}", [fo, dims[l]], F32, kind="ExternalInput"),
            nc.dram_tensor(f"spline_w{l}", [fo, dims[l], COEFF], F32, kind="ExternalInput"),
            nc.dram_tensor(f"scaler{l}", [fo, dims[l]], F32, kind="ExternalInput"),
        ))
    reg_w = nc.dram_tensor("reg_w", [1, WIDTHS[-1]], F32, kind="ExternalInput")
    reg_b = nc.dram_tensor("reg_b", [1], F32, kind="ExternalInput")
    aux_w = nc.dram_tensor("aux_w", [1, WIDTHS[-1]], F32, kind="ExternalInput")
    aux_b = nc.dram_tensor("aux_b", [1], F32, kind="ExternalInput")
    out = nc.dram_tensor("out", [2, B], F32, kind="ExternalOutput")
    dbg = nc.dram_tensor("dbg", [128, B], F32, kind="ExternalOutput")

    with tile.TileContext(nc) as tc:
        with tc.tile_pool(name="consts", bufs=1) as consts, \
             tc.tile_pool(name="hp", bufs=16) as hp, \
             tc.tile_pool(name="hacc", bufs=16) as haccp, \
             tc.tile_pool(name="rhs", bufs=16) as rhsp, \
             tc.tile_pool(name="bases", bufs=16) as basesp, \
             tc.tile_pool(name="scr", bufs=6) as scrp, \
             tc.tile_pool(name="wload", bufs=2) as wload, \
             tc.tile_pool(name="bww", bufs=2) as bwwp, \
             tc.tile_pool(name="scload", bufs=2) as scload, \
             tc.tile_pool(name="sw", bufs=2) as swp, \
             tc.tile_pool(name="wT", bufs=3) as wTp, \
             tc.tile_pool(name="wT8", bufs=7) as wT8p, \
             tc.tile_pool(name="scT", bufs=2) as scTp, \
             tc.tile_pool(name="psA", bufs=4, space="PSUM") as psA, \
             tc.tile_pool(name="psT", bufs=4, space="PSUM") as psT:

            ident = consts.tile([128, 128], F16, tag="ident")
            make_identity(nc, ident)
            mb_sb = consts.tile([128, 32], F32, tag="mb")
            nc.sync.dma_start(mb_sb, mult_b[:].rearrange("(t p) -> p t", p=128))
            hw_sb = consts.tile([2, 1024], F16, tag="hw")
            nc.gpsimd.dma_start(hw_sb[0:1, :], reg_w[:])
            nc.gpsimd.dma_start(hw_sb[1:2, :], aux_w[:])
            hb_sb = consts.tile([2, 1], F32, tag="hb")
            nc.sync.dma_start(hb_sb[0:1, :], reg_b[None, :])
            nc.sync.dma_start(hb_sb[1:2, :], aux_b[None, :])
            ubias = consts.tile([128, COEFF], F32, tag="ubias")
            for c in range(COEFF):
                nc.vector.memset(ubias[:, c:c + 1], BIAS_C[c])

            # ---- KAN weight loads (shared by phase_b + startup prefetch) ----
            def load_weights(l, hf, o):
                bw_d, sw_d, sc_d = kan[l]
                fi = dims[l]
                ISUBS = fi // 256
                f0 = hf * ISUBS * 128
                osl = slice(o * 128, (o + 1) * 128)
                bwt = bwwp.tile([128, ISUBS * 128], F16, tag="bww")
                nc.gpsimd.dma_start(bwt, bw_d[osl, f0:f0 + ISUBS * 128])
                sct = scload.tile([128, ISUBS * 128], F16, tag="sc")
                nc.gpsimd.dma_start(sct, sc_d[osl, f0:f0 + ISUBS * 128])
                swts = []
                for ic in range(ISUBS * 128 // CH):
                    swt = swp.tile([128, CH * COEFF], F16, tag="sw")
                    nc.gpsimd.dma_start(
                        swt, sw_d[osl, f0 + ic * CH:f0 + (ic + 1) * CH, :]
                        .rearrange("o i c -> o (i c)"))
                    swts.append(swt.rearrange("p (i c) -> p i c", c=COEFF))
                return bwt, sct, swts

            prefetched = {(0, 0, 0): load_weights(0, 0, 0)}

            # ---- x^T: cast x to fp16 and PE-transpose into feature-major ----
            IT0 = D // 128  # 16
            xb = []  # xb[i] [128, B] fp16, partitions = features
            for i in range(IT0):
                xb.append(haccp.tile([128, B], F16, tag="hacc", name=f"xb{i}"))
            for bt in range(B // 128):  # 4 batch tiles
                xf = wload.tile([128, D], F16, tag="wload")
                nc.gpsimd.dma_start(xf, x[bt * 128:(bt + 1) * 128, :])
                for i2 in range(0, IT0, 8):
                    pt = psT.tile([128, 1024], F16, tag="pt")
                    for q in range(8):
                        nc.tensor.transpose(pt[:, q * 128:(q + 1) * 128],
                                            xf[:, (i2 + q) * 128:(i2 + q + 1) * 128],
                                            ident)
                    for q in range(8):
                        nc.scalar.copy(xb[i2 + q][:, bt * 128:(bt + 1) * 128],
                                       pt[:, q * 128:(q + 1) * 128])

            # ---- multiplicative layer: hh = x @ mult_w.T + b; h=sig(gate)*val
            h_tiles = []
            for j in range(IT0):  # output tiles of h (2048 feats)
                sig = None
                for half, o in ((0, j), (1, j + 16)):
                    acc = psA.tile([128, B], F32, tag="acc")
                    wstrip = wload.tile([128, D], F16, tag="wload")
                    nc.gpsimd.dma_start(wstrip, mult_w[o * 128:(o + 1) * 128, :])
                    for i2 in range(0, IT0, 8):
                        pt = psT.tile([128, 1024], F16, tag="pt")
                        for q in range(8):
                            nc.tensor.transpose(
                                pt[:, q * 128:(q + 1) * 128],
                                wstrip[:, (i2 + q) * 128:(i2 + q + 1) * 128],
                                ident)
                        wT = wTp.tile([128, 1024], F16, tag="wT")
                        nc.scalar.copy(wT, pt)
                        for q in range(8):
                            i = i2 + q
                            nc.tensor.matmul(acc, wT[:, q * 128:(q + 1) * 128],
                                             xb[i], start=(i == 0),
                                             stop=(i == IT0 - 1))
                    if half == 0:
                        sig = rhsp.tile([128, B], F16, tag="rhs", name=f"sig{j}")
                        nc.scalar.activation(sig, acc, AF.Sigmoid,
                                             bias=mb_sb[:, j:j + 1])
                    else:
                        val = rhsp.tile([128, B], F16, tag="rhs", name=f"val{j}")
                        nc.vector.tensor_scalar(val, acc, mb_sb[:, 16 + j:17 + j],
                                                WS_H, OP.add, OP.mult)
                        ht = hp.tile([128, B], F16, tag="h")
                        nc.vector.tensor_tensor(ht, sig, val, OP.mult)
                        h_tiles.append(ht)

            # ---- KAN layers: 2-sweep k-split pipeline ----
            silu_t = {}
            bas_t = {}

            def phase_a_half(l, hf, h_in):
                """silu + M4 bases for i in [hf*IT/2, (hf+1)*IT/2) of layer l."""
                fi = dims[l]
                IT = fi // 128
                for i in range(hf * IT // 2, (hf + 1) * IT // 2):
                    st = rhsp.tile([128, B], F16, tag="rhs", name=f"silu{l}_{i}")
                    nc.scalar.activation(st, h_in[i], AF.Silu, scale=1.0 / WS_H)
                    silu_t[(l, i)] = st
                    # layout [bas_c0, d_c0, bas_c1, d_c1, ...]: fp8 bases plus
                    # their fp8 quantization residual, so each DoubleRow pair
                    # (bas planes / residual planes) is a strided view
                    bas = basesp.tile([128, 2 * COEFF * B], F8, tag="bases",
                                      name=f"bas{l}_{i}")
                    bas_t[(l, i)] = bas
                    bas_v = bas.rearrange("p (c two b) -> p c two b", two=2, b=B)
                    for cg in range(2):  # channel groups of 3
                        u3 = scrp.tile([128, 3 * B], F16, tag="scr", name="u3")
                        for cc in range(3):
                            c = cg * 3 + cc
                            nc.scalar.activation(u3[:, cc * B:(cc + 1) * B],
                                                 h_in[i], AF.Abs,
                                                 bias=ubias[:, c:c + 1],
                                                 scale=SC_U / WS_H)
                        a = scrp.tile([128, 3 * B], F16, tag="scr", name="a")
                        nc.vector.tensor_scalar(a, u3, 2 * ALPHA, 2 * ALPHA,
                                                OP.min, OP.subtract)
                        bq = scrp.tile([128, 3 * B], F16, tag="scr", name="bq")
                        nc.vector.tensor_scalar(bq, u3, ALPHA, ALPHA,
                                                OP.min, OP.subtract)
                        a2 = scrp.tile([128, 3 * B], F16, tag="scr", name="a2")
                        nc.vector.tensor_tensor(a2, a, a, OP.mult)
                        nc.vector.tensor_tensor(a2, a2, a, OP.mult)  # A^3
                        b2 = scrp.tile([128, 3 * B], F16, tag="scr", name="b2")
                        nc.scalar.activation(b2, bq, AF.Square, scale=2.0)  # 4B^2
                        nc.vector.tensor_tensor(b2, b2, bq, OP.mult)  # 4B^3
                        t16 = scrp.tile([128, 3 * B], F16, tag="scr", name="t16")
                        nc.gpsimd.tensor_tensor(t16, b2, a2, OP.subtract)
                        t16_v = t16.rearrange("p (c b) -> p c b", b=B)
                        q_v = bas_v[:, cg * 3:(cg + 1) * 3, 0, :]
                        d_v = bas_v[:, cg * 3:(cg + 1) * 3, 1, :]
                        nc.scalar.activation(q_v, t16_v, AF.Copy)
                        nc.vector.tensor_tensor(d_v, t16_v, q_v, OP.subtract)

            def phase_b_half(l, hf, hacc):
                """matmul sweep over k-half hf; hf=0 stashes fp16 partials,
                hf=1 adds them back and emits the layer output."""
                bw_d, sw_d, sc_d = kan[l]
                fi, fo = dims[l], dims[l + 1]
                IT, OT = fi // 128, fo // 128
                ISUBS = IT // 2  # 8 i-subtiles per half
                i0 = hf * ISUBS  # first global i-subtile
                f0 = i0 * 128    # first feature
                out_tiles = []
                for o in range(OT):
                    osl = slice(o * 128, (o + 1) * 128)
                    key = (l, hf, o)
                    if key in prefetched:
                        bwt, sct, swts = prefetched.pop(key)
                    else:
                        bwt, sct, swts = load_weights(l, hf, o)
                    # scaler transpose: [o,i] -> [i,o], one bank of 8; the WS
                    # fp8 pre-scale rides the evacuation copy
                    pt_sc = psT.tile([128, 1024], F16, tag="pt")
                    for q in range(ISUBS):
                        nc.tensor.transpose(pt_sc[:, q * 128:(q + 1) * 128],
                                            sct[:, q * 128:(q + 1) * 128], ident)
                    scT = scTp.tile([128, 1024], F16, tag="scT")
                    nc.scalar.activation(scT, pt_sc, AF.Copy, scale=WS)
                    # base-weight transposes, one bank of 8 (fp16 path)
                    pt_bw = psT.tile([128, 1024], F16, tag="pt")
                    for q in range(ISUBS):
                        nc.tensor.transpose(pt_bw[:, q * 128:(q + 1) * 128],
                                            bwt[:, q * 128:(q + 1) * 128], ident)
                    bwT = wTp.tile([128, 1024], F16, tag="wT")
                    nc.scalar.activation(bwT, pt_bw, AF.Copy, scale=WS_H)
                    # spline-weight transposes: one bank per (q-group g of 4,
                    # c-pair cp), laid out [c0 q0..3 | c1 q0..3] so the fp8
                    # DoubleRow lhsT is a plane-strided view. The evac fuses
                    # the transposed-scaler multiply and the fp8 quantization.
                    swT = {}
                    for g in range(ISUBS // 4):
                        scT_b = scT[:, g * 512:(g + 1) * 512]
                        scT_v = scT_b[:, None, :].to_broadcast((128, 2, 512))
                        for cp in range(COEFF // 2):
                            pt3 = psT.tile([128, 1024], F16, tag="pt")
                            for ci in range(2):
                                for q in range(4):
                                    isub = g * 4 + q
                                    src = swts[isub * 128 // CH]
                                    qq = isub - (isub * 128 // CH) * (CH // 128)
                                    nc.tensor.transpose(
                                        pt3[:, (ci * 4 + q) * 128:
                                            (ci * 4 + q + 1) * 128],
                                        src[:, qq * 128:(qq + 1) * 128,
                                            2 * cp + ci], ident)
                            wt = wT8p.tile([128, 1024], F8, tag="wT8")
                            nc.vector.tensor_tensor(
                                wt.rearrange("p (r c) -> p r c", r=2),
                                pt3.rearrange("p (r c) -> p r c", r=2),
                                scT_v, OP.mult)
                            swT[(g, cp)] = wt.rearrange("p (r c) -> p r c", r=2)
                    # matmul sweep: fp16 base path + fp8 DoubleRow spline path
                    # (two DR matmuls per c-pair: quantized bases + residual)
                    acc = psA.tile([128, B], F32, tag="acc")
                    n_mm = ISUBS * 7
                    k = 0
                    for q in range(ISUBS):
                        i_g = i0 + q
                        qsl = slice(q * 128, (q + 1) * 128)
                        nc.tensor.matmul(acc, bwT[:, qsl], silu_t[(l, i_g)],
                                         start=(k == 0), stop=(k == n_mm - 1))
                        k += 1
                        basg = bas_t[(l, i_g)].rearrange(
                            "p (c two b) -> p c two b", two=2, b=B)
                        g, qq = q // 4, q % 4
                        for cp in range(COEFF // 2):
                            lhs = swT[(g, cp)][:, :, qq * 128:(qq + 1) * 128]
                            for res in range(2):
                                nc.tensor.matmul(
                                    acc, lhs,
                                    basg[:, 2 * cp:2 * cp + 2, res, :],
                                    start=(k == 0), stop=(k == n_mm - 1),
                                    perf_mode=PM.DoubleRow)
                                k += 1
                    if hf == 0:
                        ht = haccp.tile([128, B], F16, tag="hacc",
                                        name=f"hacc{l}_{o}")
                        nc.scalar.copy(ht, acc)
                        out_tiles.append(ht)
                    else:
                        ht = hp.tile([128, B], F16, tag="h", name=f"h{l}_{o}")
                        nc.vector.tensor_tensor(ht, acc, hacc[o], OP.add)
                        out_tiles.append(ht)
                return out_tiles

            n_layers = min(3, max(0, stage - 1))
            if n_layers:
                phase_a_half(0, 0, h_tiles)
            cur_h = h_tiles
            for l in range(n_layers):
                phase_a_half(l, 1, cur_h)
                hacc = phase_b_half(l, 0, None)
                new_h = phase_b_half(l, 1, hacc)
                cur_h = new_h
                if l + 1 < n_layers:
                    phase_a_half(l + 1, 0, cur_h)
            h_tiles = cur_h

            # ---- debug tap: first live tile of h_tiles ----
            if stage < 5:
                dbg_t = consts.tile([128, B], F32, tag="dbgt")
                nc.vector.tensor_copy(dbg_t, h_tiles[0])
                nc.sync.dma_start(dbg[:], dbg_t)

            # ---- heads ----
            if stage >= 5:
                acc = psA.tile([128, B], F32, tag="acc")
                IT2 = WIDTHS[-1] // 128  # 8
                for i in range(IT2):
                    pt = psT.tile([128, 1024], F16, tag="pt")
                    nc.tensor.transpose(pt[:, 0:2],
                                        hw_sb[:, i * 128:(i + 1) * 128],
                                        ident[0:2, 0:2])
                    wT = wTp.tile([128, 1024], F16, tag="wT")
                    nc.scalar.copy(wT[:, 0:2], pt[:, 0:2])
                    nc.tensor.matmul(acc[0:2, :], wT[:, 0:2], h_tiles[i],
                                     start=(i == 0), stop=(i == IT2 - 1))
                res = consts.tile([2, B], F32, tag="res")
                nc.vector.tensor_scalar(res, acc[0:2, :], 1.0 / WS_H,
                                        hb_sb[:, 0:1], OP.mult, OP.add)
                nc.sync.dma_start(out[:], res)

    return _patch_json(nc)


_NC = None


def kernel(**inputs):
    global _NC
    from concourse.bass_utils import run_bass_kernel_spmd

    if _NC is None:
        _NC = build(int(os.environ.get("KSTAGE", "99")))
    per_core = []
    x_full = np.ascontiguousarray(inputs["x"], dtype=np.float32)
    shared = {k: np.ascontiguousarray(np.asarray(v), dtype=np.float32)
              for k, v in inputs.items() if k != "x"}
    for c in range(N_CORES):
        m = dict(shared)
        m["x"] = np.ascontiguousarray(x_full[c * B:(c + 1) * B])
        per_core.append(m)
    res = run_bass_kernel_spmd(_NC, per_core, core_ids=list(range(N_CORES)))
    reg = np.concatenate([res.results[c]["out"][0] for c in range(N_CORES)])
    aux = np.concatenate([res.results[c]["out"][1] for c in range(N_CORES)])
    kernel.last_results = res
    return reg, aux


# revision 32
# speedup vs baseline: 1.4348x; 1.4348x over previous
"""BRD4KANModel Trainium2 kernel, v2.

Data-parallel over batch across 8 NeuronCores (512 rows each, weights
replicated). On-chip layout is feature-major (h^T: features on partitions,
batch on the free dim); every layer's matmul output [out_feat, batch]
feeds the next layer directly. All matmul operands are fp16 (cast from
f32 during the SWDGE DMA); weights are transposed on-chip by the PE
(128x128 is_transpose matmuls into fp16 PSUM banks of 8 tiles each).

B-spline bases use the symmetric cardinal form: B_c(x) = M4(t),
t = (x - grid[c+2])/h, M4(t) = (1/6)(2-|t|)+^3 - (4/6)(1-|t|)+^3.
With U = a|t| (a = 6^(-1/3), one ACT Abs per channel), both truncated
powers come from min/subtract chains whose constants are channel-
independent, so all DVE steps run batched over the 6 channels at fp16
2x/4x DVE rates:
    A  = min(U, 2a) - 2a          (= -a(2-|t|)+,  tensor_scalar 4x)
    Bq = min(U, a)  - a           (= -a(1-|t|)+,  tensor_scalar 4x)
    A3 = (A*A)*A,  B3 = (2Bq)^2*Bq  (squares+cubes, tensor_tensor 2x)
    t16 = B3 - A3                 (on Pool; it is idle otherwise)

The spline path (6/7 of the MACs) runs as fp8e4 DoubleRow matmuls (2
K-planes per instruction at 0.5 cycles/row). The bases are stored as
fp8 plus their fp8 quantization residual, interleaved so each DoubleRow
c-pair is one strided view; each c-pair issues two DoubleRow matmuls
(quantized + residual) sharing one stationary bank, which restores
near-fp16 bases accuracy while halving spline PE time. The base path
(silu @ base_w) carries h's dominant signal and stays fp16. End-to-end
max-abs relative error ~1.5e-2 (gate: 2e-2), dominated by the one-shot
e4m3 quantization of the scaled spline weights.

The per-(out,in) spline scaler multiply rides the PSUM->SBUF evacuation
of the weight transposes: the scaler strip is itself PE-transposed to
[in, out] layout, and the spline-weight evac becomes one tensor_tensor
multiply per PSUM bank (fused with the fp8 quantization) instead of a
separate full pass over all 63M spline weights.

This walrus build accepts only ONE semaphore wait per instruction, while
Tile's scheduler attaches several; _split_waits() post-processes the BIR
JSON, hoisting excess waits onto NoOps inserted just before each
instruction on the same engine.
"""

import json
import os

import numpy as np

import concourse.bass as bass
import concourse.mybir as mybir
import concourse.tile as tile
from concourse.masks import make_identity

F32 = mybir.dt.float32
F16 = mybir.dt.float16
F8 = mybir.dt.float8e4
AF = mybir.ActivationFunctionType
OP = mybir.AluOpType
PM = mybir.MatmulPerfMode

# fp8 weight pre-scale: spline weights (sw*sc ~ 2e-3 rms) sit below the
# e4m3 subnormal floor, so all KAN weights are scaled by WS at evacuation
# and h flows through the network multiplied by WS; the ACT input scales
# of silu/abs (and the head epilogue) divide it back out for free.
WS = 128.0
WS_H = WS

N_CORES = 8
BATCH = 4096
B = BATCH // N_CORES  # 512 per core
D = 2048
WIDTHS = [2048, 2048, 1024]
COEFF = 6
GRID_SIZE = 3
SPLINE_ORDER = 3
HG = 2.0 / GRID_SIZE
GRID = [m * HG - 1.0 - SPLINE_ORDER * HG for m in range(GRID_SIZE + 2 * SPLINE_ORDER + 1)]
ALPHA = 6.0 ** (-1.0 / 3.0)
SC_U = ALPHA / HG
BIAS_C = [-ALPHA * GRID[c + 2] / HG for c in range(COEFF)]

CH = 512          # spline i-chunk (features per weight DMA chunk)


def _split_waits(bir_bytes: bytes, keep: int = 1) -> bytes:
    d = json.loads(bir_bytes)
    for f in d["functions"]:
        for bb in f["blocks"]:
            new_insts = []
            for inst in bb["instructions"]:
                si = inst.get("sync_info")
                waits = (si or {}).get("on_wait") or []
                if len(waits) > keep:
                    extra = waits[:-keep]
                    inst["sync_info"]["on_wait"] = waits[-keep:]
                    for ci in range(0, len(extra), keep):
                        new_insts.append({
                            "name": f"{inst['name']}-w{ci}",
                            "opcode": "NoOp",
                            "engine": inst["engine"],
                            "ins": [],
                            "outs": [],
                            "debug": inst.get("debug"),
                            "sync_info": {"on_update": [],
                                          "on_wait": extra[ci:ci + keep]},
                        })
                new_insts.append(inst)
            bb["instructions"] = new_insts
    return json.dumps(d).encode()


def _patch_json(nc):
    orig = nc.to_json_bytes

    def patched():
        return _split_waits(orig())

    nc.to_json_bytes = patched
    return nc


def build(stage=99):
    nc = bass.Bass()
    x = nc.dram_tensor("x", [B, D], F32, kind="ExternalInput")
    mult_w = nc.dram_tensor("mult_w", [2 * D, D], F32, kind="ExternalInput")
    mult_b = nc.dram_tensor("mult_b", [2 * D], F32, kind="ExternalInput")
    kan = []
    dims = [D] + WIDTHS
    for l in range(3):
        fo = dims[l + 1]
        kan.append((
            nc.dram_tensor(f"base_w{l}", [fo, dims[l]], F32, kind="ExternalInput"),
            nc.dram_tensor(f"spline_w{l}", [fo, dims[l], COEFF], F32, kind="ExternalInput"),
            nc.dram_tensor(f"scaler{l}", [fo, dims[l]], F32, kind="ExternalInput"),
        ))
    reg_w = nc.dram_tensor("reg_w", [1, WIDTHS[-1]], F32, kind="ExternalInput")
    reg_b = nc.dram_tensor("reg_b", [1], F32, kind="ExternalInput")
    aux_w = nc.dram_tensor("aux_w", [1, WIDTHS[-1]], F32, kind="ExternalInput")
    aux_b = nc.dram_tensor("aux_b", [1], F32, kind="ExternalInput")
    out = nc.dram_tensor("out", [2, B], F32, kind="ExternalOutput")
    dbg = nc.dram_tensor("dbg", [128, B], F32, kind="ExternalOutput")

    with tile.TileContext(nc) as tc:
        with tc.tile_pool(name="consts", bufs=1) as consts, \
             tc.tile_pool(name="hp", bufs=16) as hp, \
             tc.tile_pool(name="hacc", bufs=16) as haccp, \
             tc.tile_pool(name="rhs", bufs=16) as rhsp, \
             tc.tile_pool(name="bases", bufs=16) as basesp, \
             tc.tile_pool(name="scr", bufs=6) as scrp, \
             tc.tile_pool(name="wload", bufs=2) as wload, \
             tc.tile_pool(name="scload", bufs=2) as scload, \
             tc.tile_pool(name="sw", bufs=2) as swp, \
             tc.tile_pool(name="wT", bufs=3) as wTp, \
             tc.tile_pool(name="wT8", bufs=7) as wT8p, \
             tc.tile_pool(name="scT", bufs=2) as scTp, \
             tc.tile_pool(name="psA", bufs=4, space="PSUM") as psA, \
             tc.tile_pool(name="psT", bufs=4, space="PSUM") as psT:

            ident = consts.tile([128, 128], F16, tag="ident")
            make_identity(nc, ident)
            mb_sb = consts.tile([128, 32], F32, tag="mb")
            nc.sync.dma_start(mb_sb, mult_b[:].rearrange("(t p) -> p t", p=128))
            hw_sb = consts.tile([2, 1024], F16, tag="hw")
            nc.gpsimd.dma_start(hw_sb[0:1, :], reg_w[:])
            nc.gpsimd.dma_start(hw_sb[1:2, :], aux_w[:])
            hb_sb = consts.tile([2, 1], F32, tag="hb")
            nc.sync.dma_start(hb_sb[0:1, :], reg_b[None, :])
            nc.sync.dma_start(hb_sb[1:2, :], aux_b[None, :])
            ubias = consts.tile([128, COEFF], F32, tag="ubias")
            for c in range(COEFF):
                nc.vector.memset(ubias[:, c:c + 1], BIAS_C[c])

            # ---- KAN weight loads (shared by phase_b + startup prefetch) ----
            def load_weights(l, hf, o):
                bw_d, sw_d, sc_d = kan[l]
                fi = dims[l]
                ISUBS = fi // 256
                f0 = hf * ISUBS * 128
                osl = slice(o * 128, (o + 1) * 128)
                bwt = wload.tile([128, ISUBS * 128], F16, tag="wload")
                nc.gpsimd.dma_start(bwt, bw_d[osl, f0:f0 + ISUBS * 128])
                sct = scload.tile([128, ISUBS * 128], F16, tag="sc")
                nc.gpsimd.dma_start(sct, sc_d[osl, f0:f0 + ISUBS * 128])
                swts = []
                for ic in range(ISUBS * 128 // CH):
                    swt = swp.tile([128, CH * COEFF], F16, tag="sw")
                    nc.gpsimd.dma_start(
                        swt, sw_d[osl, f0 + ic * CH:f0 + (ic + 1) * CH, :]
                        .rearrange("o i c -> o (i c)"))
                    swts.append(swt.rearrange("p (i c) -> p i c", c=COEFF))
                return bwt, sct, swts

            # ---- x^T: cast x to fp16 and PE-transpose into feature-major ----
            IT0 = D // 128  # 16
            xb = []  # xb[i] [128, B] fp16, partitions = features
            for i in range(IT0):
                xb.append(haccp.tile([128, B], F16, tag="hacc", name=f"xb{i}"))
            for bt in range(B // 128):  # 4 batch tiles
                xf = wload.tile([128, D], F16, tag="wload")
                nc.gpsimd.dma_start(xf, x[bt * 128:(bt + 1) * 128, :])
                for i2 in range(0, IT0, 8):
                    pt = psT.tile([128, 1024], F16, tag="pt")
                    for q in range(8):
                        nc.tensor.transpose(pt[:, q * 128:(q + 1) * 128],
                                            xf[:, (i2 + q) * 128:(i2 + q + 1) * 128],
                                            ident)
                    for q in range(8):
                        nc.scalar.copy(xb[i2 + q][:, bt * 128:(bt + 1) * 128],
                                       pt[:, q * 128:(q + 1) * 128])

            # ---- multiplicative layer: hh = x @ mult_w.T + b; h=sig(gate)*val
            h_tiles = []
            for j in range(IT0):  # output tiles of h (2048 feats)
                sig = None
                for half, o in ((0, j), (1, j + 16)):
                    acc = psA.tile([128, B], F32, tag="acc")
                    wstrip = wload.tile([128, D], F16, tag="wload")
                    nc.gpsimd.dma_start(wstrip, mult_w[o * 128:(o + 1) * 128, :])
                    for i2 in range(0, IT0, 8):
                        pt = psT.tile([128, 1024], F16, tag="pt")
                        for q in range(8):
                            nc.tensor.transpose(
                                pt[:, q * 128:(q + 1) * 128],
                                wstrip[:, (i2 + q) * 128:(i2 + q + 1) * 128],
                                ident)
                        wT = wTp.tile([128, 1024], F16, tag="wT")
                        nc.scalar.copy(wT, pt)
                        for q in range(8):
                            i = i2 + q
                            nc.tensor.matmul(acc, wT[:, q * 128:(q + 1) * 128],
                                             xb[i], start=(i == 0),
                                             stop=(i == IT0 - 1))
                    if half == 0:
                        sig = rhsp.tile([128, B], F16, tag="rhs", name=f"sig{j}")
                        nc.scalar.activation(sig, acc, AF.Sigmoid,
                                             bias=mb_sb[:, j:j + 1])
                    else:
                        val = rhsp.tile([128, B], F16, tag="rhs", name=f"val{j}")
                        nc.vector.tensor_scalar(val, acc, mb_sb[:, 16 + j:17 + j],
                                                WS_H, OP.add, OP.mult)
                        ht = hp.tile([128, B], F16, tag="h")
                        nc.vector.tensor_tensor(ht, sig, val, OP.mult)
                        h_tiles.append(ht)

            # ---- KAN layers: 2-sweep k-split pipeline ----
            silu_t = {}
            bas_t = {}

            def phase_a_half(l, hf, h_in):
                """silu + M4 bases for i in [hf*IT/2, (hf+1)*IT/2) of layer l."""
                fi = dims[l]
                IT = fi // 128
                for i in range(hf * IT // 2, (hf + 1) * IT // 2):
                    st = rhsp.tile([128, B], F16, tag="rhs", name=f"silu{l}_{i}")
                    nc.scalar.activation(st, h_in[i], AF.Silu, scale=1.0 / WS_H)
                    silu_t[(l, i)] = st
                    # layout [bas_c0, d_c0, bas_c1, d_c1, ...]: fp8 bases plus
                    # their fp8 quantization residual, so each DoubleRow pair
                    # (bas planes / residual planes) is a strided view
                    bas = basesp.tile([128, 2 * COEFF * B], F8, tag="bases",
                                      name=f"bas{l}_{i}")
                    bas_t[(l, i)] = bas
                    bas_v = bas.rearrange("p (c two b) -> p c two b", two=2, b=B)
                    for cg in range(2):  # channel groups of 3
                        u3 = scrp.tile([128, 3 * B], F16, tag="scr", name="u3")
                        for cc in range(3):
                            c = cg * 3 + cc
                            nc.scalar.activation(u3[:, cc * B:(cc + 1) * B],
                                                 h_in[i], AF.Abs,
                                                 bias=ubias[:, c:c + 1],
                                                 scale=SC_U / WS_H)
                        a = scrp.tile([128, 3 * B], F16, tag="scr", name="a")
                        nc.vector.tensor_scalar(a, u3, 2 * ALPHA, 2 * ALPHA,
                                                OP.min, OP.subtract)
                        bq = scrp.tile([128, 3 * B], F16, tag="scr", name="bq")
                        nc.vector.tensor_scalar(bq, u3, ALPHA, ALPHA,
                                                OP.min, OP.subtract)
                        a2 = scrp.tile([128, 3 * B], F16, tag="scr", name="a2")
                        nc.vector.tensor_tensor(a2, a, a, OP.mult)
                        nc.vector.tensor_tensor(a2, a2, a, OP.mult)  # A^3
                        b2 = scrp.tile([128, 3 * B], F16, tag="scr", name="b2")
                        nc.scalar.activation(b2, bq, AF.Square, scale=2.0)  # 4B^2
                        nc.vector.tensor_tensor(b2, b2, bq, OP.mult)  # 4B^3
                        t16 = scrp.tile([128, 3 * B], F16, tag="scr", name="t16")
                        nc.gpsimd.tensor_tensor(t16, b2, a2, OP.subtract)
                        t16_v = t16.rearrange("p (c b) -> p c b", b=B)
                        q_v = bas_v[:, cg * 3:(cg + 1) * 3, 0, :]
                        d_v = bas_v[:, cg * 3:(cg + 1) * 3, 1, :]
                        nc.scalar.activation(q_v, t16_v, AF.Copy)
                        nc.vector.tensor_tensor(d_v, t16_v, q_v, OP.subtract)

            def phase_b_half(l, hf, hacc):
                """matmul sweep over k-half hf; hf=0 stashes fp16 partials,
                hf=1 adds them back and emits the layer output."""
                bw_d, sw_d, sc_d = kan[l]
                fi, fo = dims[l], dims[l + 1]
                IT, OT = fi // 128, fo // 128
                ISUBS = IT // 2  # 8 i-subtiles per half
                i0 = hf * ISUBS  # first global i-subtile
                f0 = i0 * 128    # first feature
                out_tiles = []
                for o in range(OT):
                    osl = slice(o * 128, (o + 1) * 128)
                    bwt, sct, swts = load_weights(l, hf, o)
                    # scaler transpose: [o,i] -> [i,o], one bank of 8; the WS
                    # fp8 pre-scale rides the evacuation copy
                    pt_sc = psT.tile([128, 1024], F16, tag="pt")
                    for q in range(ISUBS):
                        nc.tensor.transpose(pt_sc[:, q * 128:(q + 1) * 128],
                                            sct[:, q * 128:(q + 1) * 128], ident)
                    scT = scTp.tile([128, 1024], F16, tag="scT")
                    nc.scalar.activation(scT, pt_sc, AF.Copy, scale=WS)
                    # base-weight transposes, one bank of 8 (fp16 path)
                    pt_bw = psT.tile([128, 1024], F16, tag="pt")
                    for q in range(ISUBS):
                        nc.tensor.transpose(pt_bw[:, q * 128:(q + 1) * 128],
                                            bwt[:, q * 128:(q + 1) * 128], ident)
                    bwT = wTp.tile([128, 1024], F16, tag="wT")
                    nc.scalar.activation(bwT, pt_bw, AF.Copy, scale=WS_H)
                    # spline-weight transposes: one bank per (q-group g of 4,
                    # c-pair cp), laid out [c0 q0..3 | c1 q0..3] so the fp8
                    # DoubleRow lhsT is a plane-strided view. The evac fuses
                    # the transposed-scaler multiply and the fp8 quantization.
                    swT = {}
                    for g in range(ISUBS // 4):
                        scT_b = scT[:, g * 512:(g + 1) * 512]
                        scT_v = scT_b[:, None, :].to_broadcast((128, 2, 512))
                        for cp in range(COEFF // 2):
                            pt3 = psT.tile([128, 1024], F16, tag="pt")
                            for ci in range(2):
                                for q in range(4):
                                    isub = g * 4 + q
                                    src = swts[isub * 128 // CH]
                                    qq = isub - (isub * 128 // CH) * (CH // 128)
                                    nc.tensor.transpose(
                                        pt3[:, (ci * 4 + q) * 128:
                                            (ci * 4 + q + 1) * 128],
                                        src[:, qq * 128:(qq + 1) * 128,
                                            2 * cp + ci], ident)
                            wt = wT8p.tile([128, 1024], F8, tag="wT8")
                            nc.vector.tensor_tensor(
                                wt.rearrange("p (r c) -> p r c", r=2),
                                pt3.rearrange("p (r c) -> p r c", r=2),
                                scT_v, OP.mult)
                            swT[(g, cp)] = wt.rearrange("p (r c) -> p r c", r=2)
                    # matmul sweep: fp16 base path + fp8 DoubleRow spline path
                    # (two DR matmuls per c-pair: quantized bases + residual)
                    acc = psA.tile([128, B], F32, tag="acc")
                    n_mm = ISUBS * 7
                    k = 0
                    for q in range(ISUBS):
                        i_g = i0 + q
                        qsl = slice(q * 128, (q + 1) * 128)
                        nc.tensor.matmul(acc, bwT[:, qsl], silu_t[(l, i_g)],
                                         start=(k == 0), stop=(k == n_mm - 1))
                        k += 1
                        basg = bas_t[(l, i_g)].rearrange(
                            "p (c two b) -> p c two b", two=2, b=B)
                        g, qq = q // 4, q % 4
                        for cp in range(COEFF // 2):
                            lhs = swT[(g, cp)][:, :, qq * 128:(qq + 1) * 128]
                            for res in range(2):
                                nc.tensor.matmul(
                                    acc, lhs,
                                    basg[:, 2 * cp:2 * cp + 2, res, :],
                                    start=(k == 0), stop=(k == n_mm - 1),
                                    perf_mode=PM.DoubleRow)
                                k += 1
                    if hf == 0:
                        ht = haccp.tile([128, B], F16, tag="hacc",
                                        name=f"hacc{l}_{o}")
                        nc.scalar.copy(ht, acc)
                        out_tiles.append(ht)
                    else:
                        ht = hp.tile([128, B], F16, tag="h", name=f"h{l}_{o}")
                        nc.vector.tensor_tensor(ht, acc, hacc[o], OP.add)
                        out_tiles.append(ht)
                return out_tiles

            n_layers = min(3, max(0, stage - 1))
            if n_layers:
                phase_a_half(0, 0, h_tiles)
            cur_h = h_tiles
            for l in range(n_layers):
                phase_a_half(l, 1, cur_h)
                hacc = phase_b_half(l, 0, None)
                new_h = phase_b_half(l, 1, hacc)
                cur_h = new_h
                if l + 1 < n_layers:
                    phase_a_half(l + 1, 0, cur_h)
            h_tiles = cur_h

            # ---- debug tap: first live tile of h_tiles ----
            if stage < 5:
                dbg_t = consts.tile([128, B], F32, tag="dbgt")
                nc.vector.tensor_copy(dbg_t, h_tiles[0])
                nc.sync.dma_start(dbg[:], dbg_t)

            # ---- heads ----
            if stage >= 5:
                acc = psA.tile([128, B], F32, tag="acc")
                IT2 = WIDTHS[-1] // 128  # 8
                for i in range(IT2):
                    pt = psT.tile([128, 1024], F16, tag="pt")
                    nc.tensor.transpose(pt[:, 0:2],
                                        hw_sb[:, i * 128:(i + 1) * 128],
                                        ident[0:2, 0:2])
                    wT = wTp.tile([128, 1024], F16, tag="wT")
                    nc.scalar.copy(wT[:, 0:2], pt[:, 0:2])
                    nc.tensor.matmul(acc[0:2, :], wT[:, 0:2], h_tiles[i],
                                     start=(i == 0), stop=(i == IT2 - 1))
                res = consts.tile([2, B], F32, tag="res")
                nc.vector.tensor_scalar(res, acc[0:2, :], 1.0 / WS_H,
                                        hb_sb[:, 0:1], OP.mult, OP.add)
                nc.sync.dma_start(out[:], res)

    return _patch_json(nc)


_NC = None


def kernel(**inputs):
    global _NC
    from concourse.bass_utils import run_bass_kernel_spmd

    if _NC is None:
        _NC = build(int(os.environ.get("KSTAGE", "99")))
    per_core = []
    x_full = np.ascontiguousarray(inputs["x"], dtype=np.float32)
    shared = {k: np.ascontiguousarray(np.asarray(v), dtype=np.float32)
              for k, v in inputs.items() if k != "x"}
    for c in range(N_CORES):
        m = dict(shared)
        m["x"] = np.ascontiguousarray(x_full[c * B:(c + 1) * B])
        per_core.append(m)
    res = run_bass_kernel_spmd(_NC, per_core, core_ids=list(range(N_CORES)))
    reg = np.concatenate([res.results[c]["out"][0] for c in range(N_CORES)])
    aux = np.concatenate([res.results[c]["out"][1] for c in range(N_CORES)])
    kernel.last_results = res
    return reg, aux
